# revision 35
# baseline (speedup 1.0000x reference)
"""Expert-parallel MoE MLP kernel for Trainium2 (8 NeuronCores).

Problem: out[b,e,n,d] = gelu(x[b,e] @ w1[e] + b1[e]) @ w2[e] + b2[e]
Shapes: x [2,8,1024,1024] f32, w1 [8,1024,4096], b1 [8,4096],
        w2 [8,4096,1024], b2 [8,1024].

Sharding: expert e -> core e. Each core runs a 2048-token MLP:
  [2048,1024] @ [1024,4096] -> gelu -> @ [4096,1024] -> [2048,1024]

Device-side layout: activations live transposed ([feature, token]) so the
contraction dim is always the SBUF partition dim:
  phase 1: psum[h_tile, t] += w1[d_tile, h_tile].T @ xT[d_tile, t]
  phase 2: psum[d_tile, t] += w2[h_tile, d_tile].T @ hT[h_tile, t]
Host transposes x on the way in and out on the way back (part of
shard/unshard), so the device does zero transposes.

All matmul inputs are bf16 (fp32 PSUM accumulation); GELU (tanh approx,
matching jax.nn.gelu default) fused with the b1 add on ScalarE.
"""

import sys

for _p in ("/opt/trn_rl_repo",):
    if _p not in sys.path:
        sys.path.insert(0, _p)

import numpy as np
import ml_dtypes

from contextlib import ExitStack

import concourse.bass as bass
import concourse.tile as tile
from concourse import bacc, mybir
from concourse.bass import _add_dep_helper
from concourse.bass_utils import run_bass_kernel_spmd

BF16 = mybir.dt.bfloat16
F32 = mybir.dt.float32

# Full-problem constants (hardcoded per harness contract).
B, E, N, D, H = 2, 8, 1024, 1024, 4096
T = B * N          # tokens per expert/core
TBLK = 512         # tokens per block (= one PSUM bank of fp32)
P = 128


def build_nc(t=T, d=D, h=H, tblk=TBLK, act=None, repeats=1,
             ps_bufs=2, act_mode="gelu", phases=(1, 2), x_mode="stream",
             chain_pe=False, warm=7):
    """Build the per-core Bass program. All cores run this same program on
    different data (SPMD). repeats>1 re-runs the token-block loop (weights
    stay resident) — used only for steady-state timing measurements.
    act_mode: "gelu" | "copy_dve" (diagnostic: replace gelu w/ DVE copy)."""
    if act is None:
        act = mybir.ActivationFunctionType.Gelu_apprx_tanh
    kd = d // P        # contraction tiles for phase 1
    nh = h // P        # h tiles (phase-1 outputs / phase-2 contraction)
    nd = d // P        # d tiles (phase-2 outputs)
    nblk = t // tblk

    nc = bacc.Bacc("TRN2", target_bir_lowering=False)

    # x and w1 come in host-pre-tiled so each DMA moves large contiguous
    # runs per partition row (8KB / 2KB): the natural row-major slices
    # produced 256B packets, and the DMA engines are packet-rate-bound
    # (~30ns/packet), throttling the kernel head to ~140GB/s.
    xt_hbm = nc.dram_tensor(
        "xt", [nblk, P, kd * tblk], BF16, kind="ExternalInput").ap()
    w1_hbm = nc.dram_tensor(
        "w1", [nh, P, kd * P], BF16, kind="ExternalInput").ap()
    w2_hbm = nc.dram_tensor("w2", [h, d], BF16, kind="ExternalInput").ap()
    b1_hbm = nc.dram_tensor("b1", [P, nh], F32, kind="ExternalInput").ap()
    b2_hbm = nc.dram_tensor("b2", [P, nd], F32, kind="ExternalInput").ap()
    out_hbm = nc.dram_tensor("outT", [d, t], F32, kind="ExternalOutput").ap()

    xt_bv = [xt_hbm[ib].rearrange("p (k tt) -> p k tt", k=kd)
             for ib in range(nblk)]
    w1_tv = [w1_hbm[ih].rearrange("p (k j) -> p k j", k=kd)
             for ih in range(nh)]
    w2_v = w2_hbm.rearrange("(kh p) d -> p kh d", p=P)

    with tile.TileContext(nc) as tc, ExitStack() as ctx:
        w1_pool = ctx.enter_context(tc.tile_pool(name="w1", bufs=nh))
        w2_pool = ctx.enter_context(tc.tile_pool(name="w2", bufs=nh))
        # bufs=1 on x: block ib+1's DMA then waits until block ib's tile is
        # fully consumed by phase 1, keeping the 1MB transfer out of the
        # bandwidth-critical kernel head (it has a ~50us idle window).
        x_pool = ctx.enter_context(tc.tile_pool(name="x", bufs=1))
        h_pool = ctx.enter_context(tc.tile_pool(name="h", bufs=nh + 2))
        o_pool = ctx.enter_context(tc.tile_pool(name="o", bufs=4))
        c_pool = ctx.enter_context(tc.tile_pool(name="c", bufs=1))
        ps1 = ctx.enter_context(
            tc.tile_pool(name="ps1", bufs=ps_bufs + 2, space="PSUM"))
        ps2 = ctx.enter_context(tc.tile_pool(name="ps2", bufs=ps_bufs, space="PSUM"))

        # PE clock warm-up: the PE p-state needs ~3us of continuous busy to
        # reach full clock. Run a few dummy matmuls on scratch data while the
        # head DMAs are in flight so the real matmuls start at full speed.
        if warm:
            wm_pool = ctx.enter_context(tc.tile_pool(name="wm", bufs=1))
            ps_w = ctx.enter_context(tc.tile_pool(name="psw", bufs=1, space="PSUM"))
            wm_w = wm_pool.tile([P, P], BF16)
            wm_x = wm_pool.tile([P, tblk], BF16)
            nc.vector.memset(wm_w, 0)
            nc.vector.memset(wm_x, 0)
            wm_ps = ps_w.tile([P, tblk], F32)
            for _ in range(warm):
                nc.tensor.matmul(wm_ps, wm_w, wm_x, start=True, stop=True)

        # Block-0 input, first: the kernel head is HBM-bandwidth-bound, so
        # block 0's 1MB is split into 4 chunks. The earliest-consumed chunks
        # ride the scalar queue (hardware DGE, fast start); the rest ride
        # gpsimd (software DGE, ~3us slower first byte). w1 owns sync.
        xt0 = None
        if x_mode == "stream" and 1 in phases:
            xt0 = x_pool.tile([P, kd, tblk], BF16)
            nq = 4 if kd % 4 == 0 else (2 if kd % 2 == 0 else 1)
            q = kd // nq
            # b1 rides first (tiny transfer): the first gelu needs it, and a
            # late first gelu stalls the phase-1 PSUM ring.
            b1_sb = c_pool.tile([P, nh], F32)
            nc.scalar.dma_start(out=b1_sb, in_=b1_hbm)
            for ic in range(nq):
                nc.scalar.dma_start(out=xt0[:, ic * q:(ic + 1) * q, :],
                                    in_=xt_bv[0][:, ic * q:(ic + 1) * q, :])
        else:
            b1_sb = c_pool.tile([P, nh], F32)
            nc.scalar.dma_start(out=b1_sb, in_=b1_hbm)

        # Biases pre-transposed on host to [P, n] so the DMA is contiguous —
        # a transposing 4B-element DMA here used to block the sync queue
        # (and thus all w1 loads) for ~10us at kernel start.
        b2_sb = c_pool.tile([P, nd], F32)
        nc.scalar.dma_start(out=b2_sb, in_=b2_hbm)

        # Weights, resident in SBUF for the whole kernel. Chunked DMAs so
        # compute can start as soon as the first chunks land; the first tile
        # is split in two so the very first matmul waits on a half-size DMA.
        w1_t = []
        for ih in range(nh):
            wt = w1_pool.tile([P, kd, P], BF16)
            if ih == 0:
                h2 = kd // 2
                nc.sync.dma_start(out=wt[:, :h2, :], in_=w1_tv[0][:, :h2, :])
                nc.sync.dma_start(out=wt[:, h2:, :], in_=w1_tv[0][:, h2:, :])
            else:
                nc.sync.dma_start(out=wt, in_=w1_tv[ih])
            w1_t.append(wt)
        w2_t = []
        for ikh in range(nh):
            wt = w2_pool.tile([P, d], BF16)
            nc.sync.dma_start(out=wt, in_=w2_v[:, ikh, :])
            w2_t.append(wt)

        prev_mm = [None]

        def MM(*args, **kwargs):
            bi = nc.tensor.matmul(*args, **kwargs)
            if chain_pe and prev_mm[0] is not None:
                _add_dep_helper(bi.ins, prev_mm[0].ins, sync=False,
                                reason="pe emission order")
            prev_mm[0] = bi
            return bi

        gelu = act
        gelu_anchor = [None]
        xt_pre = {}
        if x_mode == "preload":
            for ib in range(nblk):
                xt_pre[ib] = c_pool.tile([P, kd, tblk], BF16,
                                         name=f"xp{ib}", tag=f"xp{ib}")
                nc.sync.dma_start(out=xt_pre[ib], in_=xt_bv[ib])
        for ib in [i % nblk for i in range(nblk * repeats)]:
            tsl = slice(ib * tblk, (ib + 1) * tblk)
            if x_mode == "preload":
                xt = xt_pre[ib]
            elif x_mode == "stream" and ib == 0 and xt0 is not None:
                xt = xt0
            else:
                xt = x_pool.tile([P, kd, tblk], BF16)
                if x_mode == "hwdge":
                    bi = nc.sync.dma_start(out=xt, in_=xt_bv[ib])
                else:
                    bi = nc.gpsimd.dma_start(out=xt, in_=xt_bv[ib])
                if gelu_anchor[0] is not None:
                    # Keep the next block's 1MB prefetch out of the
                    # bandwidth-critical kernel head: it is only needed
                    # ~110us later, but with no dep it fires at ~9us and
                    # starves the w1/x-block-0 streams the PE is waiting on.
                    _add_dep_helper(bi.ins, gelu_anchor[0].ins, sync=True,
                                    reason="delay x prefetch past head")

            # phase 1: hT[h_tile] = gelu(w1.T @ xT + b1)
            def p1_act(ps, ih):
                hs = h_pool.tile([P, tblk], BF16)
                if act_mode == "gelu":
                    abi = nc.scalar.activation(
                        hs, ps, gelu, bias=b1_sb[:, ih:ih + 1])
                else:
                    abi = nc.vector.tensor_copy(hs, ps)
                if ih == min(8, nh - 1):
                    gelu_anchor[0] = abi
                return hs

            ht = []
            if 1 in phases:
                ih0 = 0
                if ib == 0 and xt0 is not None and nh >= 3:
                    # Head fill is HBM-roofline-bound and x chunks arrive
                    # progressively; interleave the first three chains at
                    # ik-segment granularity so the PE consumes each x chunk
                    # as it lands instead of stalling on the full block.
                    # 3 chains x 2 ik x 216ns ~= the ~2us chunk cadence.
                    S = 3
                    segsz = 2 if kd % 2 == 0 else 1
                    ps_l = [ps1.tile([P, tblk], F32, name="ps", tag="ps")
                            for _ in range(S)]
                    for s0 in range(0, kd, segsz):
                        for ch in range(S):
                            for ik in range(s0, s0 + segsz):
                                MM(
                                    ps_l[ch], w1_t[ch][:, ik, :], xt[:, ik, :],
                                    start=(ik == 0), stop=(ik == kd - 1),
                                )
                    for ch in range(S):
                        ht.append(p1_act(ps_l[ch], ch))
                    ih0 = S
                for ih in range(ih0, nh):
                    ps = ps1.tile([P, tblk], F32, name="ps", tag="ps")
                    for ik in range(kd):
                        MM(
                            ps, w1_t[ih][:, ik, :], xt[:, ik, :],
                            start=(ik == 0), stop=(ik == kd - 1),
                        )
                    ht.append(p1_act(ps, ih))
            else:
                # diagnostic: fake hT from xt slices (kd divides nh usage)
                for ih in range(nh):
                    hs = h_pool.tile([P, tblk], BF16)
                    nc.vector.tensor_copy(hs, xt[:, ih % kd, :])
                    ht.append(hs)

            # phase 2: outT[d_tile] = w2.T @ hT + b2
            if 2 in phases:
                for idt in range(nd):
                    # The very last d-tile of the last block is computed in
                    # two column halves so the kernel-final output DMA is
                    # half-size and overlaps the second half's matmuls.
                    split = 2 if (ib == nblk - 1 and idt == nd - 1) else 1
                    cw = tblk // split
                    for ic in range(split):
                        csl = slice(ic * cw, (ic + 1) * cw)
                        ps = ps2.tile([P, cw], F32)
                        for ikh in range(nh):
                            MM(
                                ps, w2_t[ikh][:, idt * P:(idt + 1) * P],
                                ht[ikh][:, csl],
                                start=(ikh == 0), stop=(ikh == nh - 1),
                            )
                        ob = o_pool.tile([P, cw], F32)
                        nc.vector.tensor_scalar_add(ob, ps, b2_sb[:, idt:idt + 1])
                        nc.scalar.dma_start(
                            out=out_hbm[idt * P:(idt + 1) * P,
                                        ib * tblk + ic * cw:
                                        ib * tblk + (ic + 1) * cw],
                            in_=ob,
                        )
            elif 1 in phases:
                # keep outputs observable so phase-1 work isn't dead
                idt = 0
                ob = o_pool.tile([P, tblk], F32)
                nc.vector.tensor_copy(ob, ht[ib % nh])
                nc.scalar.dma_start(
                    out=out_hbm[idt * P:(idt + 1) * P, tsl], in_=ob
                )

    nc.compile()
    return nc


_NC_CACHE = {}


def _get_nc():
    if "nc" not in _NC_CACHE:
        _NC_CACHE["nc"] = build_nc()
    return _NC_CACHE["nc"]


def pack_x(xe, t, d, tblk):
    """[t, d] activations -> [nblk, P, kd*tblk] bf16, matching the device
    tile layout so each DMA row is one long contiguous run."""
    bf16 = ml_dtypes.bfloat16
    kd, nblk = d // P, t // tblk
    a = np.asarray(xe).astype(bf16).reshape(nblk, tblk, kd, P)
    return np.ascontiguousarray(a.transpose(0, 3, 2, 1).reshape(
        nblk, P, kd * tblk))


def pack_w1(w1e, d, h):
    """[d, h] weights -> [nh, P, kd*P] bf16 (pre-tiled per h-tile)."""
    bf16 = ml_dtypes.bfloat16
    kd, nh = d // P, h // P
    a = np.asarray(w1e).astype(bf16).reshape(kd, P, nh, P)
    return np.ascontiguousarray(a.transpose(2, 1, 0, 3).reshape(
        nh, P, kd * P))


def make_in_maps(x, w1, b1, w2, b2):
    bf16 = ml_dtypes.bfloat16
    in_maps = []
    for e in range(E):
        xe = np.asarray(x[:, e], dtype=np.float32).reshape(T, D)
        in_maps.append({
            "xt": pack_x(xe, T, D, TBLK),
            "w1": pack_w1(w1[e], D, H),
            "w2": np.asarray(w2[e], dtype=np.float32).astype(bf16),
            # biases pre-transposed to [P, n] so the device DMA is contiguous
            "b1": np.ascontiguousarray(
                np.asarray(b1[e], np.float32).reshape(H // P, P).T),
            "b2": np.ascontiguousarray(
                np.asarray(b2[e], np.float32).reshape(D // P, P).T),
        })
    return in_maps


def kernel(x, w1, b1, w2, b2):
    nc = _get_nc()
    in_maps = make_in_maps(x, w1, b1, w2, b2)

    res = run_bass_kernel_spmd(nc, in_maps, core_ids=list(range(E)))

    out = np.empty((B, E, N, D), dtype=np.float32)
    for e in range(E):
        ot = np.asarray(res.results[e]["outT"])            # [D, T]
        out[:, e] = ot.T.reshape(B, N, D)
    return out



# revision 38
# speedup vs baseline: 1.0003x; 1.0003x over previous
"""Expert-parallel MoE MLP kernel for Trainium2 (8 NeuronCores).

Problem: out[b,e,n,d] = gelu(x[b,e] @ w1[e] + b1[e]) @ w2[e] + b2[e]
Shapes: x [2,8,1024,1024] f32, w1 [8,1024,4096], b1 [8,4096],
        w2 [8,4096,1024], b2 [8,1024].

Sharding: expert e -> core e. Each core runs a 2048-token MLP:
  [2048,1024] @ [1024,4096] -> gelu -> @ [4096,1024] -> [2048,1024]

Device-side layout: activations live transposed ([feature, token]) so the
contraction dim is always the SBUF partition dim:
  phase 1: psum[h_tile, t] += w1[d_tile, h_tile].T @ xT[d_tile, t]
  phase 2: psum[d_tile, t] += w2[h_tile, d_tile].T @ hT[h_tile, t]
Host transposes x on the way in and out on the way back (part of
shard/unshard), so the device does zero transposes.

All matmul inputs are bf16 (fp32 PSUM accumulation); GELU (tanh approx,
matching jax.nn.gelu default) fused with the b1 add on ScalarE.
"""

import sys

for _p in ("/opt/trn_rl_repo",):
    if _p not in sys.path:
        sys.path.insert(0, _p)

import numpy as np
import ml_dtypes

from contextlib import ExitStack

import concourse.bass as bass
import concourse.tile as tile
from concourse import bacc, mybir
from concourse.bass import _add_dep_helper
from concourse.bass_utils import run_bass_kernel_spmd

BF16 = mybir.dt.bfloat16
F32 = mybir.dt.float32

# Full-problem constants (hardcoded per harness contract).
B, E, N, D, H = 2, 8, 1024, 1024, 4096
T = B * N          # tokens per expert/core
TBLK = 512         # tokens per block (= one PSUM bank of fp32)
P = 128


def build_nc(t=T, d=D, h=H, tblk=TBLK, act=None, repeats=1,
             ps_bufs=2, act_mode="gelu", phases=(1, 2), x_mode="stream",
             chain_pe=False, warm=5):
    """Build the per-core Bass program. All cores run this same program on
    different data (SPMD). repeats>1 re-runs the token-block loop (weights
    stay resident) — used only for steady-state timing measurements.
    act_mode: "gelu" | "copy_dve" (diagnostic: replace gelu w/ DVE copy)."""
    if act is None:
        act = mybir.ActivationFunctionType.Gelu_apprx_tanh
    kd = d // P        # contraction tiles for phase 1
    nh = h // P        # h tiles (phase-1 outputs / phase-2 contraction)
    nd = d // P        # d tiles (phase-2 outputs)
    nblk = t // tblk

    nc = bacc.Bacc("TRN2", target_bir_lowering=False)

    # x and w1 come in host-pre-tiled so each DMA moves large contiguous
    # runs per partition row (8KB / 2KB): the natural row-major slices
    # produced 256B packets, and the DMA engines are packet-rate-bound
    # (~30ns/packet), throttling the kernel head to ~140GB/s.
    xt_hbm = nc.dram_tensor(
        "xt", [nblk, P, kd * tblk], BF16, kind="ExternalInput").ap()
    w1_hbm = nc.dram_tensor(
        "w1", [nh, P, kd * P], BF16, kind="ExternalInput").ap()
    w2_hbm = nc.dram_tensor("w2", [h, d], BF16, kind="ExternalInput").ap()
    b1_hbm = nc.dram_tensor("b1", [P, nh], F32, kind="ExternalInput").ap()
    b2_hbm = nc.dram_tensor("b2", [P, nd], F32, kind="ExternalInput").ap()
    out_hbm = nc.dram_tensor("outT", [d, t], F32, kind="ExternalOutput").ap()

    xt_bv = [xt_hbm[ib].rearrange("p (k tt) -> p k tt", k=kd)
             for ib in range(nblk)]
    w1_tv = [w1_hbm[ih].rearrange("p (k j) -> p k j", k=kd)
             for ih in range(nh)]
    w2_v = w2_hbm.rearrange("(kh p) d -> p kh d", p=P)

    with tile.TileContext(nc) as tc, ExitStack() as ctx:
        w1_pool = ctx.enter_context(tc.tile_pool(name="w1", bufs=nh))
        w2_pool = ctx.enter_context(tc.tile_pool(name="w2", bufs=nh))
        # bufs=1 on x: block ib+1's DMA then waits until block ib's tile is
        # fully consumed by phase 1, keeping the 1MB transfer out of the
        # bandwidth-critical kernel head (it has a ~50us idle window).
        x_pool = ctx.enter_context(tc.tile_pool(name="x", bufs=1))
        h_pool = ctx.enter_context(tc.tile_pool(name="h", bufs=nh + 2))
        o_pool = ctx.enter_context(tc.tile_pool(name="o", bufs=4))
        c_pool = ctx.enter_context(tc.tile_pool(name="c", bufs=1))
        ps1 = ctx.enter_context(
            tc.tile_pool(name="ps1", bufs=ps_bufs + 2, space="PSUM"))
        ps2 = ctx.enter_context(tc.tile_pool(name="ps2", bufs=ps_bufs, space="PSUM"))

        # PE clock warm-up: the PE p-state needs ~3us of continuous busy to
        # reach full clock. Run a few dummy matmuls on scratch data while the
        # head DMAs are in flight so the real matmuls start at full speed.
        if warm:
            wm_pool = ctx.enter_context(tc.tile_pool(name="wm", bufs=1))
            ps_w = ctx.enter_context(tc.tile_pool(name="psw", bufs=1, space="PSUM"))
            wm_w = wm_pool.tile([P, P], BF16)
            wm_x = wm_pool.tile([P, tblk], BF16)
            # parallel engines so the two memsets don't serialize
            nc.gpsimd.memset(wm_w, 0)
            nc.vector.memset(wm_x, 0)
            wm_ps = ps_w.tile([P, tblk], F32)
            for _ in range(warm):
                nc.tensor.matmul(wm_ps, wm_w, wm_x, start=True, stop=True)

        # Block-0 input, first: the kernel head is HBM-bandwidth-bound, so
        # block 0's 1MB is split into 4 chunks. The earliest-consumed chunks
        # ride the scalar queue (hardware DGE, fast start); the rest ride
        # gpsimd (software DGE, ~3us slower first byte). w1 owns sync.
        xt0 = None
        if x_mode == "stream" and 1 in phases:
            xt0 = x_pool.tile([P, kd, tblk], BF16)
            nq = 4 if kd % 4 == 0 else (2 if kd % 2 == 0 else 1)
            q = kd // nq
            # b1 rides first (tiny transfer): the first gelu needs it, and a
            # late first gelu stalls the phase-1 PSUM ring.
            b1_sb = c_pool.tile([P, nh], F32)
            nc.scalar.dma_start(out=b1_sb, in_=b1_hbm)
            for ic in range(nq):
                nc.scalar.dma_start(out=xt0[:, ic * q:(ic + 1) * q, :],
                                    in_=xt_bv[0][:, ic * q:(ic + 1) * q, :])
        else:
            b1_sb = c_pool.tile([P, nh], F32)
            nc.scalar.dma_start(out=b1_sb, in_=b1_hbm)

        # Biases pre-transposed on host to [P, n] so the DMA is contiguous —
        # a transposing 4B-element DMA here used to block the sync queue
        # (and thus all w1 loads) for ~10us at kernel start.
        b2_sb = c_pool.tile([P, nd], F32)
        nc.scalar.dma_start(out=b2_sb, in_=b2_hbm)

        # Weights, resident in SBUF for the whole kernel. Chunked DMAs so
        # compute can start as soon as the first chunks land; the first tile
        # is split in two so the very first matmul waits on a half-size DMA.
        w1_t = []
        for ih in range(nh):
            wt = w1_pool.tile([P, kd, P], BF16)
            if ih == 0:
                h2 = kd // 2
                nc.sync.dma_start(out=wt[:, :h2, :], in_=w1_tv[0][:, :h2, :])
                nc.sync.dma_start(out=wt[:, h2:, :], in_=w1_tv[0][:, h2:, :])
            else:
                nc.sync.dma_start(out=wt, in_=w1_tv[ih])
            w1_t.append(wt)
        w2_t = []
        for ikh in range(nh):
            wt = w2_pool.tile([P, d], BF16)
            nc.sync.dma_start(out=wt, in_=w2_v[:, ikh, :])
            w2_t.append(wt)

        prev_mm = [None]

        def MM(*args, **kwargs):
            bi = nc.tensor.matmul(*args, **kwargs)
            if chain_pe and prev_mm[0] is not None:
                _add_dep_helper(bi.ins, prev_mm[0].ins, sync=False,
                                reason="pe emission order")
            prev_mm[0] = bi
            return bi

        gelu = act
        gelu_anchor = [None]
        xt_pre = {}
        if x_mode == "preload":
            for ib in range(nblk):
                xt_pre[ib] = c_pool.tile([P, kd, tblk], BF16,
                                         name=f"xp{ib}", tag=f"xp{ib}")
                nc.sync.dma_start(out=xt_pre[ib], in_=xt_bv[ib])
        for ib in [i % nblk for i in range(nblk * repeats)]:
            tsl = slice(ib * tblk, (ib + 1) * tblk)
            if x_mode == "preload":
                xt = xt_pre[ib]
            elif x_mode == "stream" and ib == 0 and xt0 is not None:
                xt = xt0
            else:
                xt = x_pool.tile([P, kd, tblk], BF16)
                if x_mode == "hwdge":
                    bi = nc.sync.dma_start(out=xt, in_=xt_bv[ib])
                else:
                    bi = nc.gpsimd.dma_start(out=xt, in_=xt_bv[ib])
                if gelu_anchor[0] is not None:
                    # Keep the next block's 1MB prefetch out of the
                    # bandwidth-critical kernel head: it is only needed
                    # ~110us later, but with no dep it fires at ~9us and
                    # starves the w1/x-block-0 streams the PE is waiting on.
                    _add_dep_helper(bi.ins, gelu_anchor[0].ins, sync=True,
                                    reason="delay x prefetch past head")

            # phase 1: hT[h_tile] = gelu(w1.T @ xT + b1)
            def p1_act(ps, ih):
                hs = h_pool.tile([P, tblk], BF16)
                if act_mode == "gelu":
                    abi = nc.scalar.activation(
                        hs, ps, gelu, bias=b1_sb[:, ih:ih + 1])
                else:
                    abi = nc.vector.tensor_copy(hs, ps)
                if ih == min(8, nh - 1):
                    gelu_anchor[0] = abi
                return hs

            ht = []
            if 1 in phases:
                ih0 = 0
                if ib == 0 and xt0 is not None and nh >= 2:
                    # Head fill is HBM-roofline-bound and x chunks arrive
                    # progressively; interleave the first two chains at
                    # ik-segment granularity so the PE consumes each x chunk
                    # as it lands instead of stalling on the full block.
                    # (3 chains re-introduces the gelu-WAR psum-ring stall.)
                    S = 2
                    segsz = 2 if kd % 2 == 0 else 1
                    ps_l = [ps1.tile([P, tblk], F32, name="ps", tag="ps")
                            for _ in range(S)]
                    for s0 in range(0, kd, segsz):
                        for ch in range(S):
                            for ik in range(s0, s0 + segsz):
                                MM(
                                    ps_l[ch], w1_t[ch][:, ik, :], xt[:, ik, :],
                                    start=(ik == 0), stop=(ik == kd - 1),
                                )
                    for ch in range(S):
                        ht.append(p1_act(ps_l[ch], ch))
                    ih0 = S
                for ih in range(ih0, nh):
                    ps = ps1.tile([P, tblk], F32, name="ps", tag="ps")
                    for ik in range(kd):
                        MM(
                            ps, w1_t[ih][:, ik, :], xt[:, ik, :],
                            start=(ik == 0), stop=(ik == kd - 1),
                        )
                    ht.append(p1_act(ps, ih))
            else:
                # diagnostic: fake hT from xt slices (kd divides nh usage)
                for ih in range(nh):
                    hs = h_pool.tile([P, tblk], BF16)
                    nc.vector.tensor_copy(hs, xt[:, ih % kd, :])
                    ht.append(hs)

            # phase 2: outT[d_tile] = w2.T @ hT + b2
            if 2 in phases:
                for idt in range(nd):
                    # The very last d-tile of the last block is computed in
                    # two column halves so the kernel-final output DMA is
                    # half-size and overlaps the second half's matmuls.
                    split = 2 if (ib == nblk - 1 and idt == nd - 1) else 1
                    cw = tblk // split
                    for ic in range(split):
                        csl = slice(ic * cw, (ic + 1) * cw)
                        ps = ps2.tile([P, cw], F32)
                        for ikh in range(nh):
                            MM(
                                ps, w2_t[ikh][:, idt * P:(idt + 1) * P],
                                ht[ikh][:, csl],
                                start=(ikh == 0), stop=(ikh == nh - 1),
                            )
                        ob = o_pool.tile([P, cw], F32)
                        nc.vector.tensor_scalar_add(ob, ps, b2_sb[:, idt:idt + 1])
                        nc.scalar.dma_start(
                            out=out_hbm[idt * P:(idt + 1) * P,
                                        ib * tblk + ic * cw:
                                        ib * tblk + (ic + 1) * cw],
                            in_=ob,
                        )
            elif 1 in phases:
                # keep outputs observable so phase-1 work isn't dead
                idt = 0
                ob = o_pool.tile([P, tblk], F32)
                nc.vector.tensor_copy(ob, ht[ib % nh])
                nc.scalar.dma_start(
                    out=out_hbm[idt * P:(idt + 1) * P, tsl], in_=ob
                )

    nc.compile()
    return nc


_NC_CACHE = {}


def _get_nc():
    if "nc" not in _NC_CACHE:
        _NC_CACHE["nc"] = build_nc()
    return _NC_CACHE["nc"]


def pack_x(xe, t, d, tblk):
    """[t, d] activations -> [nblk, P, kd*tblk] bf16, matching the device
    tile layout so each DMA row is one long contiguous run."""
    bf16 = ml_dtypes.bfloat16
    kd, nblk = d // P, t // tblk
    a = np.asarray(xe).astype(bf16).reshape(nblk, tblk, kd, P)
    return np.ascontiguousarray(a.transpose(0, 3, 2, 1).reshape(
        nblk, P, kd * tblk))


def pack_w1(w1e, d, h):
    """[d, h] weights -> [nh, P, kd*P] bf16 (pre-tiled per h-tile)."""
    bf16 = ml_dtypes.bfloat16
    kd, nh = d // P, h // P
    a = np.asarray(w1e).astype(bf16).reshape(kd, P, nh, P)
    return np.ascontiguousarray(a.transpose(2, 1, 0, 3).reshape(
        nh, P, kd * P))


def make_in_maps(x, w1, b1, w2, b2):
    bf16 = ml_dtypes.bfloat16
    in_maps = []
    for e in range(E):
        xe = np.asarray(x[:, e], dtype=np.float32).reshape(T, D)
        in_maps.append({
            "xt": pack_x(xe, T, D, TBLK),
            "w1": pack_w1(w1[e], D, H),
            "w2": np.asarray(w2[e], dtype=np.float32).astype(bf16),
            # biases pre-transposed to [P, n] so the device DMA is contiguous
            "b1": np.ascontiguousarray(
                np.asarray(b1[e], np.float32).reshape(H // P, P).T),
            "b2": np.ascontiguousarray(
                np.asarray(b2[e], np.float32).reshape(D // P, P).T),
        })
    return in_maps


def kernel(x, w1, b1, w2, b2):
    nc = _get_nc()
    in_maps = make_in_maps(x, w1, b1, w2, b2)

    res = run_bass_kernel_spmd(nc, in_maps, core_ids=list(range(E)))

    out = np.empty((B, E, N, D), dtype=np.float32)
    for e in range(E):
        ot = np.asarray(res.results[e]["outT"])            # [D, T]
        out[:, e] = ot.T.reshape(B, N, D)
    return out



# revision 40
# speedup vs baseline: 1.0038x; 1.0035x over previous
"""Expert-parallel MoE MLP kernel for Trainium2 (8 NeuronCores).

Problem: out[b,e,n,d] = gelu(x[b,e] @ w1[e] + b1[e]) @ w2[e] + b2[e]
Shapes: x [2,8,1024,1024] f32, w1 [8,1024,4096], b1 [8,4096],
        w2 [8,4096,1024], b2 [8,1024].

Sharding: expert e -> core e. Each core runs a 2048-token MLP:
  [2048,1024] @ [1024,4096] -> gelu -> @ [4096,1024] -> [2048,1024]

Device-side layout: activations live transposed ([feature, token]) so the
contraction dim is always the SBUF partition dim:
  phase 1: psum[h_tile, t] += w1[d_tile, h_tile].T @ xT[d_tile, t]
  phase 2: psum[d_tile, t] += w2[h_tile, d_tile].T @ hT[h_tile, t]
Host transposes x on the way in and out on the way back (part of
shard/unshard), so the device does zero transposes.

All matmul inputs are bf16 (fp32 PSUM accumulation); GELU (tanh approx,
matching jax.nn.gelu default) fused with the b1 add on ScalarE.
"""

import sys

for _p in ("/opt/trn_rl_repo",):
    if _p not in sys.path:
        sys.path.insert(0, _p)

import numpy as np
import ml_dtypes

from contextlib import ExitStack

import concourse.bass as bass
import concourse.tile as tile
from concourse import bacc, mybir
from concourse.bass import _add_dep_helper
from concourse.bass_utils import run_bass_kernel_spmd

BF16 = mybir.dt.bfloat16
F32 = mybir.dt.float32

# Full-problem constants (hardcoded per harness contract).
B, E, N, D, H = 2, 8, 1024, 1024, 4096
T = B * N          # tokens per expert/core
TBLK = 512         # tokens per block (= one PSUM bank of fp32)
P = 128


def build_nc(t=T, d=D, h=H, tblk=TBLK, act=None, repeats=1,
             ps_bufs=2, act_mode="gelu", phases=(1, 2), x_mode="stream",
             chain_pe=False, warm=5):
    """Build the per-core Bass program. All cores run this same program on
    different data (SPMD). repeats>1 re-runs the token-block loop (weights
    stay resident) — used only for steady-state timing measurements.
    act_mode: "gelu" | "copy_dve" (diagnostic: replace gelu w/ DVE copy)."""
    if act is None:
        act = mybir.ActivationFunctionType.Gelu_apprx_tanh
    kd = d // P        # contraction tiles for phase 1
    nh = h // P        # h tiles (phase-1 outputs / phase-2 contraction)
    nd = d // P        # d tiles (phase-2 outputs)
    nblk = t // tblk

    nc = bacc.Bacc("TRN2", target_bir_lowering=False)

    # x and w1 come in host-pre-tiled so each DMA moves large contiguous
    # runs per partition row (8KB / 2KB): the natural row-major slices
    # produced 256B packets, and the DMA engines are packet-rate-bound
    # (~30ns/packet), throttling the kernel head to ~140GB/s.
    xt_hbm = nc.dram_tensor(
        "xt", [nblk, P, kd * tblk], BF16, kind="ExternalInput").ap()
    w1_hbm = nc.dram_tensor(
        "w1", [nh, P, kd * P], BF16, kind="ExternalInput").ap()
    w2_hbm = nc.dram_tensor("w2", [h, d], BF16, kind="ExternalInput").ap()
    b1_hbm = nc.dram_tensor("b1", [P, nh], F32, kind="ExternalInput").ap()
    b2_hbm = nc.dram_tensor("b2", [P, nd], F32, kind="ExternalInput").ap()
    out_hbm = nc.dram_tensor("outT", [d, t], F32, kind="ExternalOutput").ap()

    xt_bv = [xt_hbm[ib].rearrange("p (k tt) -> p k tt", k=kd)
             for ib in range(nblk)]
    w1_tv = [w1_hbm[ih].rearrange("p (k j) -> p k j", k=kd)
             for ih in range(nh)]
    w2_v = w2_hbm.rearrange("(kh p) d -> p kh d", p=P)

    with tile.TileContext(nc) as tc, ExitStack() as ctx:
        w1_pool = ctx.enter_context(tc.tile_pool(name="w1", bufs=nh))
        w2_pool = ctx.enter_context(tc.tile_pool(name="w2", bufs=nh))
        # bufs=1 on x: block ib+1's DMA then waits until block ib's tile is
        # fully consumed by phase 1, keeping the 1MB transfer out of the
        # bandwidth-critical kernel head (it has a ~50us idle window).
        x_pool = ctx.enter_context(tc.tile_pool(name="x", bufs=1))
        h_pool = ctx.enter_context(tc.tile_pool(name="h", bufs=nh + 2))
        o_pool = ctx.enter_context(tc.tile_pool(name="o", bufs=4))
        c_pool = ctx.enter_context(tc.tile_pool(name="c", bufs=1))
        ps1 = ctx.enter_context(
            tc.tile_pool(name="ps1", bufs=ps_bufs + 3, space="PSUM"))
        ps2 = ctx.enter_context(tc.tile_pool(name="ps2", bufs=ps_bufs, space="PSUM"))

        # PE clock warm-up: the PE p-state needs ~3us of continuous busy to
        # reach full clock. Run a few dummy matmuls on scratch data while the
        # head DMAs are in flight so the real matmuls start at full speed.
        if warm:
            wm_pool = ctx.enter_context(tc.tile_pool(name="wm", bufs=1))
            ps_w = ctx.enter_context(tc.tile_pool(name="psw", bufs=1, space="PSUM"))
            wm_w = wm_pool.tile([P, P], BF16)
            wm_x = wm_pool.tile([P, tblk], BF16)
            # parallel engines so the two memsets don't serialize
            nc.gpsimd.memset(wm_w, 0)
            nc.vector.memset(wm_x, 0)
            wm_ps = ps_w.tile([P, tblk], F32)
            for _ in range(warm):
                nc.tensor.matmul(wm_ps, wm_w, wm_x, start=True, stop=True)

        # Block-0 input, first: the kernel head is HBM-bandwidth-bound, so
        # block 0's 1MB is split into 4 chunks. The earliest-consumed chunks
        # ride the scalar queue (hardware DGE, fast start); the rest ride
        # gpsimd (software DGE, ~3us slower first byte). w1 owns sync.
        xt0 = None
        if x_mode == "stream" and 1 in phases:
            xt0 = x_pool.tile([P, kd, tblk], BF16)
            nq = 4 if kd % 4 == 0 else (2 if kd % 2 == 0 else 1)
            q = kd // nq
            # b1 rides first (tiny transfer): the first gelu needs it, and a
            # late first gelu stalls the phase-1 PSUM ring.
            b1_sb = c_pool.tile([P, nh], F32)
            nc.scalar.dma_start(out=b1_sb, in_=b1_hbm)
            for ic in range(nq):
                nc.scalar.dma_start(out=xt0[:, ic * q:(ic + 1) * q, :],
                                    in_=xt_bv[0][:, ic * q:(ic + 1) * q, :])
        else:
            b1_sb = c_pool.tile([P, nh], F32)
            nc.scalar.dma_start(out=b1_sb, in_=b1_hbm)

        # Biases pre-transposed on host to [P, n] so the DMA is contiguous —
        # a transposing 4B-element DMA here used to block the sync queue
        # (and thus all w1 loads) for ~10us at kernel start.
        b2_sb = c_pool.tile([P, nd], F32)
        nc.scalar.dma_start(out=b2_sb, in_=b2_hbm)

        # Weights, resident in SBUF for the whole kernel. Chunked DMAs so
        # compute can start as soon as the first chunks land; the first tile
        # is split in two so the very first matmul waits on a half-size DMA.
        w1_t = []
        for ih in range(nh):
            wt = w1_pool.tile([P, kd, P], BF16)
            if ih == 0:
                h2 = kd // 2
                nc.sync.dma_start(out=wt[:, :h2, :], in_=w1_tv[0][:, :h2, :])
                nc.sync.dma_start(out=wt[:, h2:, :], in_=w1_tv[0][:, h2:, :])
            else:
                nc.sync.dma_start(out=wt, in_=w1_tv[ih])
            w1_t.append(wt)
        w2_t = []
        for ikh in range(nh):
            wt = w2_pool.tile([P, d], BF16)
            nc.sync.dma_start(out=wt, in_=w2_v[:, ikh, :])
            w2_t.append(wt)

        prev_mm = [None]

        def MM(*args, **kwargs):
            bi = nc.tensor.matmul(*args, **kwargs)
            if chain_pe and prev_mm[0] is not None:
                _add_dep_helper(bi.ins, prev_mm[0].ins, sync=False,
                                reason="pe emission order")
            prev_mm[0] = bi
            return bi

        gelu = act
        gelu_anchor = [None]
        xt_pre = {}
        if x_mode == "preload":
            for ib in range(nblk):
                xt_pre[ib] = c_pool.tile([P, kd, tblk], BF16,
                                         name=f"xp{ib}", tag=f"xp{ib}")
                nc.sync.dma_start(out=xt_pre[ib], in_=xt_bv[ib])
        for ib in [i % nblk for i in range(nblk * repeats)]:
            tsl = slice(ib * tblk, (ib + 1) * tblk)
            if x_mode == "preload":
                xt = xt_pre[ib]
            elif x_mode == "stream" and ib == 0 and xt0 is not None:
                xt = xt0
            else:
                xt = x_pool.tile([P, kd, tblk], BF16)
                if x_mode == "hwdge":
                    bi = nc.sync.dma_start(out=xt, in_=xt_bv[ib])
                else:
                    bi = nc.gpsimd.dma_start(out=xt, in_=xt_bv[ib])
                if gelu_anchor[0] is not None:
                    # Keep the next block's 1MB prefetch out of the
                    # bandwidth-critical kernel head: it is only needed
                    # ~110us later, but with no dep it fires at ~9us and
                    # starves the w1/x-block-0 streams the PE is waiting on.
                    _add_dep_helper(bi.ins, gelu_anchor[0].ins, sync=True,
                                    reason="delay x prefetch past head")

            # phase 1: hT[h_tile] = gelu(w1.T @ xT + b1)
            def p1_act(ps, ih):
                hs = h_pool.tile([P, tblk], BF16)
                if act_mode == "gelu":
                    abi = nc.scalar.activation(
                        hs, ps, gelu, bias=b1_sb[:, ih:ih + 1])
                else:
                    abi = nc.vector.tensor_copy(hs, ps)
                if ih == min(8, nh - 1):
                    gelu_anchor[0] = abi
                return hs

            ht = []
            if 1 in phases:
                ih0 = 0
                if ib == 0 and xt0 is not None and nh >= 3:
                    # Head fill is HBM-roofline-bound and x chunks arrive
                    # progressively; interleave the first three chains at
                    # ik-segment granularity so the PE consumes each x chunk
                    # as it lands instead of stalling on the full block.
                    # Needs the 5-deep ps1 ring: with 4, chain 4 hits a
                    # gelu-WAR stall waiting for the first segment's drain.
                    S = 3
                    segsz = 2 if kd % 2 == 0 else 1
                    ps_l = [ps1.tile([P, tblk], F32, name="ps", tag="ps")
                            for _ in range(S)]
                    for s0 in range(0, kd, segsz):
                        for ch in range(S):
                            for ik in range(s0, s0 + segsz):
                                MM(
                                    ps_l[ch], w1_t[ch][:, ik, :], xt[:, ik, :],
                                    start=(ik == 0), stop=(ik == kd - 1),
                                )
                    for ch in range(S):
                        ht.append(p1_act(ps_l[ch], ch))
                    ih0 = S
                for ih in range(ih0, nh):
                    ps = ps1.tile([P, tblk], F32, name="ps", tag="ps")
                    for ik in range(kd):
                        MM(
                            ps, w1_t[ih][:, ik, :], xt[:, ik, :],
                            start=(ik == 0), stop=(ik == kd - 1),
                        )
                    ht.append(p1_act(ps, ih))
            else:
                # diagnostic: fake hT from xt slices (kd divides nh usage)
                for ih in range(nh):
                    hs = h_pool.tile([P, tblk], BF16)
                    nc.vector.tensor_copy(hs, xt[:, ih % kd, :])
                    ht.append(hs)

            # phase 2: outT[d_tile] = w2.T @ hT + b2
            if 2 in phases:
                for idt in range(nd):
                    # The very last d-tile of the last block is computed in
                    # two column halves so the kernel-final output DMA is
                    # half-size and overlaps the second half's matmuls.
                    split = 2 if (ib == nblk - 1 and idt == nd - 1) else 1
                    cw = tblk // split
                    for ic in range(split):
                        csl = slice(ic * cw, (ic + 1) * cw)
                        ps = ps2.tile([P, cw], F32)
                        for ikh in range(nh):
                            MM(
                                ps, w2_t[ikh][:, idt * P:(idt + 1) * P],
                                ht[ikh][:, csl],
                                start=(ikh == 0), stop=(ikh == nh - 1),
                            )
                        ob = o_pool.tile([P, cw], F32)
                        nc.vector.tensor_scalar_add(ob, ps, b2_sb[:, idt:idt + 1])
                        nc.scalar.dma_start(
                            out=out_hbm[idt * P:(idt + 1) * P,
                                        ib * tblk + ic * cw:
                                        ib * tblk + (ic + 1) * cw],
                            in_=ob,
                        )
            elif 1 in phases:
                # keep outputs observable so phase-1 work isn't dead
                idt = 0
                ob = o_pool.tile([P, tblk], F32)
                nc.vector.tensor_copy(ob, ht[ib % nh])
                nc.scalar.dma_start(
                    out=out_hbm[idt * P:(idt + 1) * P, tsl], in_=ob
                )

    nc.compile()
    return nc


_NC_CACHE = {}


def _get_nc():
    if "nc" not in _NC_CACHE:
        _NC_CACHE["nc"] = build_nc()
    return _NC_CACHE["nc"]


def pack_x(xe, t, d, tblk):
    """[t, d] activations -> [nblk, P, kd*tblk] bf16, matching the device
    tile layout so each DMA row is one long contiguous run."""
    bf16 = ml_dtypes.bfloat16
    kd, nblk = d // P, t // tblk
    a = np.asarray(xe).astype(bf16).reshape(nblk, tblk, kd, P)
    return np.ascontiguousarray(a.transpose(0, 3, 2, 1).reshape(
        nblk, P, kd * tblk))


def pack_w1(w1e, d, h):
    """[d, h] weights -> [nh, P, kd*P] bf16 (pre-tiled per h-tile)."""
    bf16 = ml_dtypes.bfloat16
    kd, nh = d // P, h // P
    a = np.asarray(w1e).astype(bf16).reshape(kd, P, nh, P)
    return np.ascontiguousarray(a.transpose(2, 1, 0, 3).reshape(
        nh, P, kd * P))


def make_in_maps(x, w1, b1, w2, b2):
    bf16 = ml_dtypes.bfloat16
    in_maps = []
    for e in range(E):
        xe = np.asarray(x[:, e], dtype=np.float32).reshape(T, D)
        in_maps.append({
            "xt": pack_x(xe, T, D, TBLK),
            "w1": pack_w1(w1[e], D, H),
            "w2": np.asarray(w2[e], dtype=np.float32).astype(bf16),
            # biases pre-transposed to [P, n] so the device DMA is contiguous
            "b1": np.ascontiguousarray(
                np.asarray(b1[e], np.float32).reshape(H // P, P).T),
            "b2": np.ascontiguousarray(
                np.asarray(b2[e], np.float32).reshape(D // P, P).T),
        })
    return in_maps


def kernel(x, w1, b1, w2, b2):
    nc = _get_nc()
    in_maps = make_in_maps(x, w1, b1, w2, b2)

    res = run_bass_kernel_spmd(nc, in_maps, core_ids=list(range(E)))

    out = np.empty((B, E, N, D), dtype=np.float32)
    for e in range(E):
        ot = np.asarray(res.results[e]["outT"])            # [D, T]
        out[:, e] = ot.T.reshape(B, N, D)
    return out

